# revision 6
# baseline (speedup 1.0000x reference)
"""Trainium2 Bass kernel for nn_Connection_v5 (geodesic-spray-style RHS).

Math (per sample n, D=128, 2D=256):
    x = input_[:, :D], v = input_[:, D:]
    z1 = x @ W1.T + b1            [2D]
    h  = relu(z1), mask = z1 > 0  [2D]
    s  = sigmoid(h @ W2.T + b2)   [D]
    sign_j = -1 if j < 4 else 1
    g  = (s + 0.618) * sign;  jac[i,j] = sign_i s_i(1-s_i) * (W2 (mask*W1))[i,j]
    dv[j] = -1/g_j * sum_i v_i^2 jac[i,j] + 2 v_j / g_j * sum_i v_i jac[j,i]
    out = [v, dv]

Folded form used here (signs/scales pushed into host-precomputed weights):
    nsps = (s-1)*s            (= -s(1-s))
    gr   = 1/(s+0.618)
    u    = v @ W1.T                       ; mu = mask * u
    wt   = v^2 * nsps                     ; at = wt @ (sign_i*W2) ; am = mask * at
    At   = am @ (W1*sign_j)               ; Ct = mu @ (-2*W2.T)
    dv   = gr*At + (v*nsps*gr)*Ct

Sharding: pure data-parallel over N=8192 across 8 cores (1024 rows each);
weights replicated. On-chip layout is feature-major [feat, n]; sample-major
<->feature-major conversion via PE transposes with an identity matrix.
Precision: M1 (z1, decides the relu mask) in full fp32; the other five
matmuls in bf16 (fp32 PSUM accumulate); final combine in fp32.
"""

import os
import numpy as np

D = 128
TWO_D = 256
N_TOTAL = 8192
NCORES = 8
N_CORE = N_TOTAL // NCORES  # 1024
NF = 256                    # samples per pipeline chunk (matmul moving dim)
CONST = 0.618
SIGN = 4

_CACHE = {}


def _build(n_core=N_CORE):
    """Build + compile the per-core Bass module (cached)."""
    from contextlib import ExitStack

    import concourse.bacc as bacc
    import concourse.mybir as mybir
    import concourse.tile as tile

    f32 = mybir.dt.float32
    bf16 = mybir.dt.bfloat16
    Act = mybir.ActivationFunctionType
    Op = mybir.AluOpType

    nchunk = n_core // NF
    nb = NF // 128  # 128-row blocks per chunk

    nc = bacc.Bacc("TRN2", target_bir_lowering=False, debug=False,
                   num_devices=NCORES)

    inp = nc.dram_tensor("inp", [n_core, TWO_D], f32, kind="ExternalInput").ap()
    w1t = nc.dram_tensor("w1t", [D, TWO_D], f32, kind="ExternalInput").ap()
    w1tb = nc.dram_tensor("w1tb", [D, TWO_D], bf16, kind="ExternalInput").ap()
    w2t = nc.dram_tensor("w2t", [TWO_D, D], bf16, kind="ExternalInput").ap()
    w2sgn = nc.dram_tensor("w2sgn", [D, TWO_D], bf16,
                           kind="ExternalInput").ap()
    w1sgn = nc.dram_tensor("w1sgn", [TWO_D, D], bf16,
                           kind="ExternalInput").ap()
    w2t2 = nc.dram_tensor("w2t2", [TWO_D, D], bf16, kind="ExternalInput").ap()
    b1d = nc.dram_tensor("b1d", [D, 2], f32, kind="ExternalInput").ap()
    b2d = nc.dram_tensor("b2d", [D, 1], f32, kind="ExternalInput").ap()
    idn = nc.dram_tensor("idn", [128, 128], f32, kind="ExternalInput").ap()
    out = nc.dram_tensor("out", [n_core, TWO_D], f32, kind="ExternalOutput").ap()

    with tile.TileContext(nc) as tc:
        with ExitStack() as ctx:
            singles = ctx.enter_context(tc.tile_pool(name="singles", bufs=1))
            io = ctx.enter_context(tc.tile_pool(name="io", bufs=3))
            acts = ctx.enter_context(tc.tile_pool(name="acts", bufs=2))
            psum = ctx.enter_context(
                tc.tile_pool(name="psum", bufs=8, space="PSUM"))

            sb_w1t = singles.tile([128, TWO_D], f32, name="sb_w1t")
            nc.sync.dma_start(out=sb_w1t, in_=w1t)
            sb_w1tb = singles.tile([128, TWO_D], bf16, name="sb_w1tb")
            nc.sync.dma_start(out=sb_w1tb, in_=w1tb)
            sb_w2t = singles.tile([128, 2, D], bf16, name="sb_w2t")
            nc.sync.dma_start(out=sb_w2t,
                              in_=w2t.rearrange("(c p) m -> p c m", p=128))
            sb_w2sgn = singles.tile([128, TWO_D], bf16, name="sb_w2sgn")
            nc.sync.dma_start(out=sb_w2sgn, in_=w2sgn)
            sb_w1sgn = singles.tile([128, 2, D], bf16, name="sb_w1sgn")
            nc.sync.dma_start(out=sb_w1sgn,
                              in_=w1sgn.rearrange("(c p) m -> p c m", p=128))
            sb_w2t2 = singles.tile([128, 2, D], bf16, name="sb_w2t2")
            nc.sync.dma_start(out=sb_w2t2,
                              in_=w2t2.rearrange("(c p) m -> p c m", p=128))
            sb_b1 = singles.tile([128, 2], f32, name="sb_b1")
            nc.sync.dma_start(out=sb_b1, in_=b1d)
            sb_b2 = singles.tile([128, 1], f32, name="sb_b2")
            nc.sync.dma_start(out=sb_b2, in_=b2d)
            sb_id = singles.tile([128, 128], f32, name="sb_id")
            nc.sync.dma_start(out=sb_id, in_=idn)

            inp_v = inp.rearrange("(c b p) d -> c p b d", p=128, b=nb)
            outv_v = out[:, 0:D].rearrange("(c b p) d -> c p b d", p=128, b=nb)
            outd_v = out[:, D:TWO_D].rearrange("(c b p) d -> c p b d",
                                               p=128, b=nb)

            for c in range(nchunk):
                inb = io.tile([128, nb, TWO_D], f32, tag="inb")
                nc.sync.dma_start(out=inb, in_=inp_v[c])
                # v passes through unchanged as out[:, :D]
                nc.sync.dma_start(out=outv_v[c], in_=inb[:, :, D:TWO_D])

                # sample-major -> feature-major via PE transposes
                ps_tr = psum.tile([128, 2, NF], f32, tag="ps", name="ps_tr")
                for b in range(nb):
                    nc.tensor.transpose(ps_tr[:, 0, 128 * b:128 * (b + 1)],
                                        inb[:, b, 0:D], sb_id)
                    nc.tensor.transpose(ps_tr[:, 1, 128 * b:128 * (b + 1)],
                                        inb[:, b, D:TWO_D], sb_id)
                xT = acts.tile([128, NF], f32, tag="xT")
                nc.scalar.copy(out=xT, in_=ps_tr[:, 0, :])
                # vT in fp32 (for the fp32 elementwise chain)
                vT = acts.tile([128, NF], f32, tag="vT")
                nc.scalar.copy(out=vT, in_=ps_tr[:, 1, :])
                # bf16 copy of vT for the M3 matmul
                vTb = acts.tile([128, NF], bf16, tag="vTb")
                nc.gpsimd.tensor_copy(out=vTb, in_=vT)

                # M1: z1^T = W1 @ x^T (full fp32: mask depends on its sign)
                ps_z1 = psum.tile([128, 2, NF], f32, tag="ps", name="ps_z1")
                for k in range(2):
                    nc.tensor.matmul(ps_z1[:, k, :],
                                     sb_w1t[:, 128 * k:128 * (k + 1)], xT,
                                     start=True, stop=True)
                h = acts.tile([128, 2, NF], bf16, tag="h")
                for k in range(2):
                    nc.scalar.activation(out=h[:, k, :], in_=ps_z1[:, k, :],
                                         func=Act.Relu,
                                         bias=sb_b1[:, k:k + 1], scale=1.0)

                # M3: u^T = W1 @ v^T (bf16)
                ps_u = psum.tile([128, 2, NF], f32, tag="ps", name="ps_u")
                for k in range(2):
                    nc.tensor.matmul(ps_u[:, k, :],
                                     sb_w1tb[:, 128 * k:128 * (k + 1)],
                                     vTb, start=True, stop=True)

                # M2: z2 = W2 @ h (accumulate over the two 128-row k-chunks)
                ps_z2 = psum.tile([128, NF], f32, tag="ps", name="ps_z2")
                for k in range(2):
                    nc.tensor.matmul(ps_z2, sb_w2t[:, k, :], h[:, k, :],
                                     start=(k == 0), stop=(k == 1))
                s = acts.tile([128, NF], f32, tag="s")
                nc.scalar.activation(out=s, in_=ps_z2, func=Act.Sigmoid,
                                     bias=sb_b2[:, 0:1], scale=1.0)

                gs = acts.tile([128, NF], f32, tag="gs")
                nc.gpsimd.tensor_scalar_add(gs, s, CONST)
                gr = acts.tile([128, NF], f32, tag="gr")
                nc.vector.reciprocal_approx_fast(out=gr, in_=gs)
                nsps = acts.tile([128, NF], f32, tag="nsps")
                nc.vector.scalar_tensor_tensor(out=nsps, in0=s, scalar=-1.0,
                                               in1=s, op0=Op.add, op1=Op.mult)
                v2 = acts.tile([128, NF], f32, tag="v2")
                nc.gpsimd.tensor_tensor(v2, vT, vT, Op.mult)
                wt = acts.tile([128, NF], bf16, tag="wt")
                nc.gpsimd.tensor_tensor(wt, v2, nsps, Op.mult)
                qt = acts.tile([128, NF], f32, tag="qt")
                nc.vector.tensor_tensor(qt, nsps, gr, Op.mult)
                vq = acts.tile([128, NF], f32, tag="vq")
                nc.vector.tensor_tensor(vq, vT, qt, Op.mult)

                # M4: at^T, contraction over i with (sign_i*W2)
                ps_a = psum.tile([128, 2, NF], f32, tag="ps", name="ps_a")
                for k in range(2):
                    nc.tensor.matmul(ps_a[:, k, :],
                                     sb_w2sgn[:, 128 * k:128 * (k + 1)],
                                     wt, start=True, stop=True)

                # mask-mul drains: mu = (h>0)*u, am = (h>0)*at
                mu = acts.tile([128, 2, NF], bf16, tag="mu")
                am = acts.tile([128, 2, NF], bf16, tag="am")
                for k in range(2):
                    nc.vector.scalar_tensor_tensor(
                        out=mu[:, k, :], in0=h[:, k, :], scalar=0.0,
                        in1=ps_u[:, k, :], op0=Op.is_gt, op1=Op.mult)
                    nc.vector.scalar_tensor_tensor(
                        out=am[:, k, :], in0=h[:, k, :], scalar=0.0,
                        in1=ps_a[:, k, :], op0=Op.is_gt, op1=Op.mult)

                # M5: At = am @ (W1*sign_j);  M6: Ct = mu @ (-2*W2.T)
                ps_A = psum.tile([128, NF], f32, tag="ps", name="ps_A")
                for k in range(2):
                    nc.tensor.matmul(ps_A, sb_w1sgn[:, k, :], am[:, k, :],
                                     start=(k == 0), stop=(k == 1))
                ps_C = psum.tile([128, NF], f32, tag="ps", name="ps_C")
                for k in range(2):
                    nc.tensor.matmul(ps_C, sb_w2t2[:, k, :], mu[:, k, :],
                                     start=(k == 0), stop=(k == 1))

                rA = acts.tile([128, NF], f32, tag="rA")
                nc.vector.tensor_tensor(rA, gr, ps_A, Op.mult)
                t2 = acts.tile([128, NF], f32, tag="t2")
                nc.vector.tensor_tensor(t2, vq, ps_C, Op.mult)
                dvT = acts.tile([128, NF], f32, tag="dvT")
                nc.vector.tensor_tensor(dvT, rA, t2, Op.add)

                # feature-major -> sample-major and store
                ps_dv = psum.tile([128, NF], f32, tag="ps", name="ps_dv")
                for b in range(nb):
                    nc.tensor.transpose(ps_dv[:, 128 * b:128 * (b + 1)],
                                        dvT[:, 128 * b:128 * (b + 1)], sb_id)
                ob = io.tile([128, nb, D], f32, tag="ob")
                nc.scalar.copy(out=ob, in_=ps_dv.rearrange(
                    "p (b d) -> p b d", b=nb))
                nc.sync.dma_start(out=outd_v[c], in_=ob)

    nc.compile()
    return nc


def _get_nc(n_core=N_CORE):
    key = ("nc", n_core)
    if key not in _CACHE:
        _CACHE[key] = _build(n_core)
    return _CACHE[key]


def _host_weights(W1, b1, W2, b2):
    import ml_dtypes

    W1 = np.asarray(W1, np.float32)
    b1 = np.asarray(b1, np.float32)
    W2 = np.asarray(W2, np.float32)
    b2 = np.asarray(b2, np.float32)
    bf16 = ml_dtypes.bfloat16
    sign = np.where(np.arange(D) < SIGN, -1.0, 1.0).astype(np.float32)
    return {
        "w1t": np.ascontiguousarray(W1.T),                           # [D, 2D]
        "w1tb": np.ascontiguousarray(W1.T).astype(bf16),             # [D, 2D]
        "w2t": np.ascontiguousarray(W2.T).astype(bf16),              # [2D, D]
        "w2sgn": np.ascontiguousarray(W2 * sign[:, None]).astype(bf16),
        "w1sgn": np.ascontiguousarray(W1 * sign[None, :]).astype(bf16),
        "w2t2": np.ascontiguousarray(-2.0 * W2.T).astype(bf16),
        "b1d": np.ascontiguousarray(b1.reshape(2, 128).T),           # [128, 2]
        "b2d": np.ascontiguousarray(b2.reshape(128, 1)),             # [128, 1]
        "idn": np.eye(128, dtype=np.float32),
    }


def _run(inp_np, W1, b1, W2, b2, trace=False):
    from concourse.bass_utils import run_bass_kernel_spmd

    nc = _get_nc(N_CORE)
    wmap = _host_weights(W1, b1, W2, b2)
    in_maps = []
    for c in range(NCORES):
        m = dict(wmap)
        m["inp"] = np.ascontiguousarray(
            inp_np[c * N_CORE:(c + 1) * N_CORE], np.float32)
        in_maps.append(m)
    res = run_bass_kernel_spmd(nc, in_maps, list(range(NCORES)), trace=trace)
    out = np.concatenate([r["out"] for r in res.results], axis=0)
    return out, res


def kernel(t=None, input_=None, W1=None, b1=None, W2=None, b2=None, **kw):
    inp_np = np.ascontiguousarray(np.asarray(input_, np.float32))
    trace = bool(int(os.environ.get("KERNEL_TRACE", "0")))
    out, _ = _run(inp_np, W1, b1, W2, b2, trace=trace)
    return out


def run_traced(inputs):
    """Returns (out, exec_time_ns, trace_path). Used by test.py."""
    inp_np = np.ascontiguousarray(np.asarray(inputs["input_"], np.float32))
    out, res = _run(inp_np, inputs["W1"], inputs["b1"], inputs["W2"],
                    inputs["b2"], trace=True)
    trace_path = None
    if res.instructions_and_trace is not None:
        trace_path = res.instructions_and_trace[1]
    return out, res.exec_time_ns, trace_path
